# revision 8
# baseline (speedup 1.0000x reference)
"""Trainium2 Bass kernel for per-sample masked conv2d (dynamic weight attention conv).

out[b] = conv2d(x[b], weight * m[b], stride=1, pad=1) + bias

Strategy: pure data parallel over batch (32 samples -> 8 cores x 4 samples).
Per sample, the conv is computed as 9 shifted matmuls accumulated in PSUM:
  out[o, h, w] = sum_{kh,kw,i} mw[o,i,kh,kw] * xpad[i, h+kh, w+kw]

Key layout trick: m is pre-transposed on the HOST to [b, i, kh, kw, o] so the
device kernel's masked-weight multiply (w * m) directly produces the [i, o]
stationary layout the matmul needs -- no on-device transposes at all.
The matmul path runs in bf16 (PSUM accumulates fp32): same 1 cycle/row PE
rate as f32r, but half-cost LDWEIGHTS, half SBUF, and 2x DVE throughput.
All 7 row-groups of an output-channel block accumulate in 7 of the 8 PSUM
banks so each stationary tile is loaded exactly once per (oc, sample).
"""

import sys
from contextlib import ExitStack

for _p in ("/opt/trn_rl_repo",):
    if _p not in sys.path:
        sys.path.append(_p)

import numpy as np

import concourse.bass as bass
import concourse.mybir as mybir
import concourse.tile as tile
from concourse import bacc, bass_utils

# Enable walrus LDWEIGHTS dedup: consecutive matmuls sharing the same
# stationary weights skip the redundant weight reload.
if not getattr(bass_utils, "_ldw_opt_patched", False):
    _orig_run_command = bass_utils.run_command

    def _run_command_ldw(argv, **kwargs):
        argv = ["--enable-ldw-opt=true" if a == "--enable-ldw-opt=false" else a
                for a in argv]
        return _orig_run_command(argv, **kwargs)

    bass_utils.run_command = _run_command_ldw
    bass_utils._ldw_opt_patched = True

# Problem constants (hardcoded per contract)
B, FIN, FOUT, KK, H, W = 32, 256, 256, 3, 56, 56
N_CORES = 8
BPC = B // N_CORES          # samples per core = 4
P = 128                     # partition width
NI = FIN // P               # input-channel chunks = 2
NO = FOUT // P              # output-channel chunks = 2
HP, WP = H + 2, W + 2       # padded spatial = 58x58
RG_ROWS = 8                 # output rows per matmul group
NRG = H // RG_ROWS          # row groups = 7
NTILE = RG_ROWS * W         # moving free size = 448
KSQ = KK * KK               # 9
CFREE = KSQ * FOUT          # 2304: (kh kw o) flattened
F32 = mybir.dt.float32
F32R = mybir.dt.float32r   # full-rate fp32 matmul path (free >= 256);
                           # walrus ldw-opt rejects bf16 LDWEIGHTS, so f32r


def build_program():
    """Build the single-core Bass program (same program on all 8 cores)."""
    nc = bacc.Bacc("TRN2", target_bir_lowering=False, debug=False,
                   num_devices=N_CORES)

    x_d = nc.dram_tensor("x", [BPC, FIN, H, W], F32, kind="ExternalInput").ap()
    mt_d = nc.dram_tensor("mt", [BPC, NI, P, CFREE], F32,
                          kind="ExternalInput").ap()
    wt_d = nc.dram_tensor("wt", [NI, P, CFREE], F32, kind="ExternalInput").ap()
    b_d = nc.dram_tensor("bias", [FOUT], F32, kind="ExternalInput").ap()
    o_d = nc.dram_tensor("out", [BPC, FOUT, H, W], F32,
                         kind="ExternalOutput").ap()

    HALF = CFREE // 2                  # 1152

    with tile.TileContext(nc) as tc, ExitStack() as ctx:
        consts = ctx.enter_context(tc.tile_pool(name="consts", bufs=1))
        mt_pool = ctx.enter_context(tc.tile_pool(name="mt_pool", bufs=2 * NI))
        # dedicated [128,128] stationary tiles (walrus ldw-opt requires plain
        # full-tile LDWEIGHTS access patterns, not slices of a larger tile)
        mwt_pool = ctx.enter_context(
            tc.tile_pool(name="mwt_pool", bufs=2 * NI * KSQ * NO))
        xs_pool = ctx.enter_context(tc.tile_pool(name="xs_pool", bufs=2))
        xp_pool = ctx.enter_context(tc.tile_pool(name="xp_pool", bufs=2 * NI))
        out_pool = ctx.enter_context(tc.tile_pool(name="out_pool", bufs=2))
        acc_psum = ctx.enter_context(tc.tile_pool(name="acc_psum", bufs=8,
                                                  space="PSUM"))

        # --- constants: wT in [i, (kh kw o)] layout (scalar/Act DMA ring,
        # so it doesn't serialize with m on sync or x on gpsimd) ---
        w_tiles = []
        for icc in range(NI):
            wt = consts.tile([P, CFREE], F32, name=f"wt_{icc}", tag=f"w{icc}")
            w_tiles.append(wt)
        # first halves early (gates sample 0's masked-weight multiply)
        nc.scalar.dma_start(out=w_tiles[0][:, :HALF], in_=wt_d[0][:, :HALF])
        nc.scalar.dma_start(out=w_tiles[0][:, HALF:], in_=wt_d[0][:, HALF:])

        # bias: [128, NO] with bias_t[p, oc] = bias[oc*128 + p]
        bias_t = consts.tile([P, NO], F32, name="bias_t")
        nc.scalar.dma_start(out=bias_t, in_=b_d.rearrange("(c p) -> p c", p=P))

        x_nat = x_d.rearrange("s (c p) h w -> s c p h w", p=P)
        o_nat = o_d.rearrange("s (c p) h w -> s c p (h w)", p=P)

        for s in range(BPC):
            mwt_tiles = []
            xp_tiles = []

            def load_m(icc):
                mt = mt_pool.tile([P, CFREE], F32, name=f"mt_{s}_{icc}",
                                  tag="mt")
                for h in range(2):
                    nc.sync.dma_start(out=mt[:, h * HALF:(h + 1) * HALF],
                                      in_=mt_d[s, icc][:, h * HALF:(h + 1) * HALF])
                stats = []
                for k in range(KSQ):
                    for oc in range(NO):
                        st = mwt_pool.tile([P, P], F32R,
                                           name=f"mwt_{s}_{icc}_{k}_{oc}",
                                           tag="mwt")
                        off = k * FOUT + oc * P
                        nc.vector.tensor_mul(st, mt[:, off:off + P],
                                             w_tiles[icc][:, off:off + P])
                        stats.append(st)
                mwt_tiles.append(stats)

            def load_x(icc):
                # staging tile carries a 64-elem zero scratch at the end; all
                # xp writes are DVE copies (memset can't emit f32r); DMA is
                # contiguous for efficient descriptors, repack+round on DVE.
                xs = xs_pool.tile([P, H * W + 64], F32, name=f"xs_{s}_{icc}",
                                  tag="xs")
                nc.vector.memset(xs[:, H * W:], 0.0)
                RH = H // 2
                nc.gpsimd.dma_start(out=xs[:, :RH * W],
                                    in_=x_nat[s, icc][:, :RH, :])
                nc.gpsimd.dma_start(out=xs[:, RH * W:H * W],
                                    in_=x_nat[s, icc][:, RH:, :])
                xp = xp_pool.tile([P, HP, WP], F32R, name=f"xp_{s}_{icc}",
                                  tag="xp")
                z = xs[:, H * W:H * W + WP]
                nc.vector.tensor_copy(xp[:, 0, :], z)
                nc.vector.tensor_copy(xp[:, HP - 1, :], z)
                zc = xs[:, H * W:H * W + H].rearrange("p (h o) -> p h o", o=1)
                nc.vector.tensor_copy(xp[:, 1:HP - 1, 0:1], zc)
                nc.vector.tensor_copy(xp[:, 1:HP - 1, WP - 1:WP], zc)
                nc.vector.tensor_copy(
                    xp[:, 1:RH + 1, 1:WP - 1],
                    xs[:, :RH * W].rearrange("p (h w) -> p h w", w=W))
                nc.vector.tensor_copy(
                    xp[:, RH + 1:HP - 1, 1:WP - 1],
                    xs[:, RH * W:H * W].rearrange("p (h w) -> p h w", w=W))
                xp_tiles.append(xp)

            load_m(0)
            load_x(0)
            if s == 0:
                # stream the remaining constants behind sample 0's first loads
                nc.scalar.dma_start(out=w_tiles[1][:, :HALF],
                                    in_=wt_d[1][:, :HALF])
                nc.scalar.dma_start(out=w_tiles[1][:, HALF:],
                                    in_=wt_d[1][:, HALF:])
            load_m(1)
            load_x(1)

            # --- conv matmuls: all 7 row-groups accumulate at once (7 PSUM
            # banks), so each stationary tile is LDWEIGHTS'd exactly once ---
            n_mm = KSQ * NI
            for oc in range(NO):
                osb = out_pool.tile([P, H * W], F32, name=f"osb_{s}_{oc}",
                                    tag="osb")
                accs = [acc_psum.tile([P, NTILE], F32,
                                      name=f"acc_{s}_{oc}_{rg}", tag="acc")
                        for rg in range(NRG)]
                for idx in range(n_mm):
                    icc, k = divmod(idx, KSQ)
                    kh, kw = divmod(k, KK)
                    stat = mwt_tiles[icc][k * NO + oc]
                    for rg in range(NRG):
                        r0 = rg * RG_ROWS + kh
                        rhs = xp_tiles[icc][:, r0:r0 + RG_ROWS, kw:kw + W]
                        nc.tensor.matmul(
                            accs[rg], stat, rhs,
                            start=(idx == 0),
                            stop=(idx == n_mm - 1),
                        )
                for rg in range(NRG):
                    # drain PSUM -> SBUF with bias add (Identity act)
                    nc.scalar.add(osb[:, rg * NTILE:(rg + 1) * NTILE],
                                  accs[rg], bias_t[:, oc:oc + 1])
                # stream the output in two chunks so the DMA starts before
                # the last row-groups drain (gpsimd ring: x ring is idle by
                # the time stores happen; keeps sync ring free for m loads)
                c0 = 4 * NTILE
                nc.gpsimd.dma_start(out=o_nat[s, oc][:, :c0],
                                    in_=osb[:, :c0])
                nc.gpsimd.dma_start(out=o_nat[s, oc][:, c0:],
                                    in_=osb[:, c0:])

    nc.compile()
    return nc


def shard_inputs(x, m, weight, bias):
    """Split batch across cores; replicate weight/bias.

    m / weight are pre-transposed on the host into the [i, (kh kw o)]
    stationary layout so the device kernel needs no transposes.
    """
    x = np.ascontiguousarray(np.asarray(x, dtype=np.float32))
    m = np.asarray(m, dtype=np.float32)
    weight = np.asarray(weight, dtype=np.float32)
    bias = np.ascontiguousarray(np.asarray(bias, dtype=np.float32))
    # [B, fout, fin, kh, kw] -> [B, fin, kh, kw, fout] -> [B, NI, P, CFREE]
    mt = np.ascontiguousarray(m.transpose(0, 2, 3, 4, 1)).reshape(
        B, NI, P, CFREE)
    wt = np.ascontiguousarray(weight.transpose(1, 2, 3, 0)).reshape(
        NI, P, CFREE)
    in_maps = []
    for c in range(N_CORES):
        sl = slice(c * BPC, (c + 1) * BPC)
        in_maps.append({"x": x[sl], "mt": mt[sl], "wt": wt, "bias": bias})
    return in_maps


def kernel(x, m, weight, bias, _trace=False):
    nc = build_program()
    in_maps = shard_inputs(x, m, weight, bias)
    res = bass_utils.run_bass_kernel_spmd(
        nc, in_maps, core_ids=list(range(N_CORES)), trace=_trace
    )
    out = np.concatenate([res.results[c]["out"] for c in range(N_CORES)], axis=0)
    if _trace:
        kernel.last_results = res
    return out


# revision 9
# speedup vs baseline: 1.1530x; 1.1530x over previous
"""Trainium2 Bass kernel for per-sample masked conv2d (dynamic weight attention conv).

out[b] = conv2d(x[b], weight * m[b], stride=1, pad=1) + bias

Strategy: pure data parallel over batch (32 samples -> 8 cores x 4 samples).
Per sample, the conv is computed as 9 shifted matmuls accumulated in PSUM:
  out[o, h, w] = sum_{kh,kw,i} mw[o,i,kh,kw] * xpad[i, h+kh, w+kw]

Key layout trick: m is pre-transposed on the HOST to [b, i, kh, kw, o] so the
device kernel's masked-weight multiply (w * m) directly produces the [i, o]
stationary layout the matmul needs -- no on-device transposes at all.
The matmul path runs in bf16 (PSUM accumulates fp32): same 1 cycle/row PE
rate as f32r, but half-cost LDWEIGHTS, half SBUF, and 2x DVE throughput.
All 7 row-groups of an output-channel block accumulate in 7 of the 8 PSUM
banks so each stationary tile is loaded exactly once per (oc, sample).
"""

import sys
from contextlib import ExitStack

for _p in ("/opt/trn_rl_repo",):
    if _p not in sys.path:
        sys.path.append(_p)

import numpy as np

import concourse.bass as bass
import concourse.mybir as mybir
import concourse.tile as tile
from concourse import bacc, bass_utils

# Enable walrus LDWEIGHTS dedup: consecutive matmuls sharing the same
# stationary weights skip the redundant weight reload.
if not getattr(bass_utils, "_ldw_opt_patched", False):
    _orig_run_command = bass_utils.run_command

    def _run_command_ldw(argv, **kwargs):
        argv = ["--enable-ldw-opt=true" if a == "--enable-ldw-opt=false" else a
                for a in argv]
        return _orig_run_command(argv, **kwargs)

    bass_utils.run_command = _run_command_ldw
    bass_utils._ldw_opt_patched = True

# Problem constants (hardcoded per contract)
B, FIN, FOUT, KK, H, W = 32, 256, 256, 3, 56, 56
N_CORES = 8
BPC = B // N_CORES          # samples per core = 4
P = 128                     # partition width
NI = FIN // P               # input-channel chunks = 2
NO = FOUT // P              # output-channel chunks = 2
HP, WP = H + 2, W + 2       # padded spatial = 58x58
RG_ROWS = 8                 # output rows per matmul group
NRG = H // RG_ROWS          # row groups = 7
NTILE = RG_ROWS * W         # moving free size = 448
KSQ = KK * KK               # 9
CFREE = KSQ * FOUT          # 2304: (kh kw o) flattened
F32 = mybir.dt.float32
F32R = mybir.dt.float32r   # full-rate fp32 matmul path (free >= 256);
                           # walrus ldw-opt rejects bf16 LDWEIGHTS, so f32r


def build_program():
    """Build the single-core Bass program (same program on all 8 cores)."""
    nc = bacc.Bacc("TRN2", target_bir_lowering=False, debug=False,
                   num_devices=N_CORES)

    x_d = nc.dram_tensor("x", [BPC, FIN, H, W], F32, kind="ExternalInput").ap()
    mt_d = nc.dram_tensor("mt", [BPC, NI, P, CFREE], F32,
                          kind="ExternalInput").ap()
    wt_d = nc.dram_tensor("wt", [NI, P, CFREE], F32, kind="ExternalInput").ap()
    b_d = nc.dram_tensor("bias", [FOUT], F32, kind="ExternalInput").ap()
    o_d = nc.dram_tensor("out", [BPC, FOUT, H, W], F32,
                         kind="ExternalOutput").ap()

    HALF = CFREE // 2                  # 1152

    with tile.TileContext(nc) as tc, ExitStack() as ctx:
        consts = ctx.enter_context(tc.tile_pool(name="consts", bufs=1))
        mt_pool = ctx.enter_context(tc.tile_pool(name="mt_pool", bufs=2 * NI))
        # dedicated [128,128] stationary tiles (walrus ldw-opt requires plain
        # full-tile LDWEIGHTS access patterns, not slices of a larger tile)
        mwt_pool = ctx.enter_context(
            tc.tile_pool(name="mwt_pool", bufs=2 * NI * KSQ * NO))
        xs_pool = ctx.enter_context(tc.tile_pool(name="xs_pool", bufs=2))
        xp_pool = ctx.enter_context(tc.tile_pool(name="xp_pool", bufs=2 * NI))
        out_pool = ctx.enter_context(tc.tile_pool(name="out_pool", bufs=2))
        acc_psum = ctx.enter_context(tc.tile_pool(name="acc_psum", bufs=8,
                                                  space="PSUM"))

        # --- constants: wT in [i, (kh kw o)] layout (scalar/Act DMA ring,
        # so it doesn't serialize with m on sync or x on gpsimd) ---
        w_tiles = []
        for icc in range(NI):
            wt = consts.tile([P, CFREE], F32, name=f"wt_{icc}", tag=f"w{icc}")
            w_tiles.append(wt)
        # first halves early (gates sample 0's masked-weight multiply)
        nc.scalar.dma_start(out=w_tiles[0][:, :HALF], in_=wt_d[0][:, :HALF])
        nc.scalar.dma_start(out=w_tiles[0][:, HALF:], in_=wt_d[0][:, HALF:])

        # bias: [128, NO] with bias_t[p, oc] = bias[oc*128 + p]
        bias_t = consts.tile([P, NO], F32, name="bias_t")
        nc.scalar.dma_start(out=bias_t, in_=b_d.rearrange("(c p) -> p c", p=P))

        x_nat = x_d.rearrange("s (c p) h w -> s c p h w", p=P)
        o_nat = o_d.rearrange("s (c p) h w -> s c p (h w)", p=P)

        for s in range(BPC):
            mwt_tiles = []
            xp_tiles = []

            def load_m(icc):
                mt = mt_pool.tile([P, CFREE], F32, name=f"mt_{s}_{icc}",
                                  tag="mt")
                for h in range(2):
                    nc.sync.dma_start(out=mt[:, h * HALF:(h + 1) * HALF],
                                      in_=mt_d[s, icc][:, h * HALF:(h + 1) * HALF])
                stats = []
                for k in range(KSQ):
                    for oc in range(NO):
                        st = mwt_pool.tile([P, P], F32R,
                                           name=f"mwt_{s}_{icc}_{k}_{oc}",
                                           tag="mwt")
                        off = k * FOUT + oc * P
                        nc.vector.tensor_mul(st, mt[:, off:off + P],
                                             w_tiles[icc][:, off:off + P])
                        stats.append(st)
                mwt_tiles.append(stats)

            def load_x(icc):
                # staging tile carries a 64-elem zero scratch at the end; all
                # xp writes are DVE copies (memset can't emit f32r); DMA is
                # contiguous for efficient descriptors, repack+round on DVE.
                xs = xs_pool.tile([P, H * W + 64], F32, name=f"xs_{s}_{icc}",
                                  tag="xs")
                nc.vector.memset(xs[:, H * W:], 0.0)
                RH = H // 2
                nc.gpsimd.dma_start(out=xs[:, :RH * W],
                                    in_=x_nat[s, icc][:, :RH, :])
                nc.gpsimd.dma_start(out=xs[:, RH * W:H * W],
                                    in_=x_nat[s, icc][:, RH:, :])
                xp = xp_pool.tile([P, HP, WP], F32R, name=f"xp_{s}_{icc}",
                                  tag="xp")
                z = xs[:, H * W:H * W + WP]
                nc.vector.tensor_copy(xp[:, 0, :], z)
                nc.vector.tensor_copy(xp[:, HP - 1, :], z)
                zc = xs[:, H * W:H * W + H].rearrange("p (h o) -> p h o", o=1)
                nc.vector.tensor_copy(xp[:, 1:HP - 1, 0:1], zc)
                nc.vector.tensor_copy(xp[:, 1:HP - 1, WP - 1:WP], zc)
                nc.vector.tensor_copy(
                    xp[:, 1:RH + 1, 1:WP - 1],
                    xs[:, :RH * W].rearrange("p (h w) -> p h w", w=W))
                nc.vector.tensor_copy(
                    xp[:, RH + 1:HP - 1, 1:WP - 1],
                    xs[:, RH * W:H * W].rearrange("p (h w) -> p h w", w=W))
                xp_tiles.append(xp)

            load_m(0)
            load_x(0)
            if s == 0:
                # stream the remaining constants behind sample 0's first loads
                nc.scalar.dma_start(out=w_tiles[1][:, :HALF],
                                    in_=wt_d[1][:, :HALF])
                nc.scalar.dma_start(out=w_tiles[1][:, HALF:],
                                    in_=wt_d[1][:, HALF:])
            load_m(1)
            load_x(1)

            # --- conv matmuls: all 7 row-groups accumulate at once (7 PSUM
            # banks), so each stationary tile is LDWEIGHTS'd exactly once ---
            n_mm = KSQ * NI
            for oc in range(NO):
                osb = out_pool.tile([P, H * W], F32, name=f"osb_{s}_{oc}",
                                    tag="osb")
                for block in ((0, 1, 2), (3, 4, 5), (6,)):
                    accs = {rg: acc_psum.tile([P, NTILE], F32,
                                              name=f"acc_{s}_{oc}_{rg}",
                                              tag="acc")
                            for rg in block}
                    for idx in range(n_mm):
                        icc, k = divmod(idx, KSQ)
                        kh, kw = divmod(k, KK)
                        stat = mwt_tiles[icc][k * NO + oc]
                        for rg in block:
                            r0 = rg * RG_ROWS + kh
                            rhs = xp_tiles[icc][:, r0:r0 + RG_ROWS, kw:kw + W]
                            nc.tensor.matmul(
                                accs[rg], stat, rhs,
                                start=(idx == 0),
                                stop=(idx == n_mm - 1),
                            )
                    for rg in block:
                        # drain PSUM -> SBUF with bias add (Identity act)
                        nc.scalar.add(osb[:, rg * NTILE:(rg + 1) * NTILE],
                                      accs[rg], bias_t[:, oc:oc + 1])
                    # stream the output per block so the final DMA mostly
                    # hides under remaining matmuls
                    lo, hi = block[0] * NTILE, (block[-1] + 1) * NTILE
                    hi = min(hi, H * W)
                    nc.gpsimd.dma_start(out=o_nat[s, oc][:, lo:hi],
                                        in_=osb[:, lo:hi])

    nc.compile()
    return nc


def shard_inputs(x, m, weight, bias):
    """Split batch across cores; replicate weight/bias.

    m / weight are pre-transposed on the host into the [i, (kh kw o)]
    stationary layout so the device kernel needs no transposes.
    """
    x = np.ascontiguousarray(np.asarray(x, dtype=np.float32))
    m = np.asarray(m, dtype=np.float32)
    weight = np.asarray(weight, dtype=np.float32)
    bias = np.ascontiguousarray(np.asarray(bias, dtype=np.float32))
    # [B, fout, fin, kh, kw] -> [B, fin, kh, kw, fout] -> [B, NI, P, CFREE]
    mt = np.ascontiguousarray(m.transpose(0, 2, 3, 4, 1)).reshape(
        B, NI, P, CFREE)
    wt = np.ascontiguousarray(weight.transpose(1, 2, 3, 0)).reshape(
        NI, P, CFREE)
    in_maps = []
    for c in range(N_CORES):
        sl = slice(c * BPC, (c + 1) * BPC)
        in_maps.append({"x": x[sl], "mt": mt[sl], "wt": wt, "bias": bias})
    return in_maps


def kernel(x, m, weight, bias, _trace=False):
    nc = build_program()
    in_maps = shard_inputs(x, m, weight, bias)
    res = bass_utils.run_bass_kernel_spmd(
        nc, in_maps, core_ids=list(range(N_CORES)), trace=_trace
    )
    out = np.concatenate([res.results[c]["out"] for c in range(N_CORES)], axis=0)
    if _trace:
        kernel.last_results = res
    return out
